# revision 19
# baseline (speedup 1.0000x reference)
"""Trainium2 Bass kernel for nn_MixedLoraModel_734.

Computes, for T=8192 tokens, D=4096:
    out = x @ W_base^T + b_base + scaling[token_lora][:,None] * lora(x)
where lora(x)[t] = WB[l_t] @ (WA[l_t] @ x[t]),  l_t = token_lora[t],
L=8 adapters of rank R=16 (so the full adapter stack is 8*16 = 128 rows).

Strategy (8 NeuronCores, data-parallel over tokens), v2:
  - Each core gets a 1024-token shard of x / token_lora plus replicated
    W_base, b_base, WA, WB, scaling.
  - Transposed-output formulation: for each 128-wide o-chunk `oc`,
        outT[o, t] = sum_d WT[d, o] * xT[d, t]  (+ LoRA + bias)
    with the W tile as the PE *stationary* operand and a 512-token-wide
    xT slab as the *moving* operand.  Both operands are bf16 (PE full
    rate, half the SBUF traffic); PSUM accumulates in f32.
  - Routing is dense, no data-dependent control flow:
        u_mT[j, t] = mask[j,t] * sum_d WA_flat[j, d] x[t, d],  j = 16l+r
    and one extra 128-contraction matmul per o-chunk accumulates
        sum_j u_mT[j, t] * (scaling[j//16] * WB[j//16, o, j%16])
    into the same PSUM tile as the base matmul.
  - Per-oc eviction adds the bias on the *scalar* engine (bias lives on
    the partition axis here), then the PE transposes the [o, t] tile
    back to [t, o] (pipelined one oc behind the matmuls) and a cheap
    engine copy + one 3D DMA per half writes natural-layout output.
  - All on-chip transposes are PE transposes in float32r (1.5 cy/row);
    evictions cast to bf16 where the data feeds the PE again.
  - DMA issue queues are balanced: the strided WB gathers go out on the
    scalar queue so the sync queue stays free for the W/x/out streams.
"""

import numpy as np

import concourse.bass as bass
import concourse.mybir as mybir
import concourse.tile as tile
from concourse import bacc
from concourse.masks import make_identity

P = 128
D = 4096          # d_in
O = 4096          # d_out
NCORES = 8
T = 8192
TS = T // NCORES  # 1024 tokens per core
NT = TS // P      # 8 token tiles per core
ND = D // P       # 32 contraction chunks
NOC = O // P      # 32 output chunks of 128
HT = TS // 2      # 512-wide moving halves
L, R, LR = 8, 16, 128

F32 = mybir.dt.float32
F32R = mybir.dt.float32r
BF16 = mybir.dt.bfloat16
I32 = mybir.dt.int32
EQ = mybir.AluOpType.is_equal
MUL = mybir.AluOpType.mult
IDENT = mybir.ActivationFunctionType.Identity


def _build() -> bass.Bass:
    nc = bacc.Bacc(None)

    x = nc.declare_dram_parameter("x", [TS, D], F32R, isOutput=False)
    w = nc.declare_dram_parameter("w", [O, D], F32R, isOutput=False)
    b = nc.declare_dram_parameter("b", [O], F32, isOutput=False)
    wa = nc.declare_dram_parameter("wa", [LR, D], F32R, isOutput=False)
    wb = nc.declare_dram_parameter("wb", [L, O, R], F32R, isOutput=False)
    scal = nc.declare_dram_parameter("scal", [L], F32, isOutput=False)
    tl = nc.declare_dram_parameter("tl", [TS], I32, isOutput=False)
    out = nc.declare_dram_parameter("out", [TS, O], F32, isOutput=True)

    with tile.TileContext(nc) as tc:
        with (
            tc.tile_pool(name="const", bufs=1) as const,
            tc.tile_pool(name="res", bufs=1) as res,
        ):
            ident_f = const.tile([P, P], F32)
            make_identity(nc, ident_f)
            ident = const.tile([P, P], F32R)
            nc.vector.tensor_copy(ident[:], ident_f[:])
            ones_row = const.tile([1, P], F32)
            nc.vector.memset(ones_row[:], 1.0)
            jdiv16 = const.tile([P, 1], F32)
            scal16 = const.tile([P, 1], F32)
            ball = const.tile([P, NOC], F32)

            # Resident bf16 operand stacks (PE inputs):
            # xT:   chunk dc in cols [dc*TS, (dc+1)*TS); xT[p, dc*TS+t] =
            #       x[t, dc*128+p]
            # waT:  waT[p, dc*128 + j] = WA_flat[j, dc*128+p]
            # wbsT: wbsT[j, o] = scaling[j//16] * WB[j//16, o, j%16]
            # u_mT: u_mT[j, t] = masked, per-token-selected  x[t] @ WA[l_t]^T
            xT = res.tile([P, ND * TS], BF16, tag="xT")
            waT = res.tile([P, D], BF16, tag="waT")
            wbsT = res.tile([P, O], BF16, tag="wbsT")
            u_mT = res.tile([P, TS], BF16, tag="u_mT")

            # ---------------- prepass ----------------
            # Phase S: index/scaling/bias columns.
            with tc.tile_pool(name="preS", bufs=1) as preS:
                psS_cm = tc.tile_pool(name="psS", bufs=1, space="PSUM")
                psS = psS_cm.__enter__()
                irow_i = preS.tile([1, P], I32, tag="iri")
                nc.gpsimd.iota(irow_i[:], pattern=[[1, L], [0, R]], base=0,
                               channel_multiplier=0)
                irow_f = preS.tile([1, P], F32, tag="irf")
                nc.vector.tensor_copy(irow_f[:], irow_i[:])
                pcol = psS.tile([P, 1], F32, tag="pcol")
                nc.tensor.matmul(pcol[:], irow_f[:], ones_row[0:1, 0:1],
                                 start=True, stop=True)
                nc.vector.tensor_copy(jdiv16[:], pcol[:])

                # scal16[p] = scaling[p//16] via E[l, j] = (j//16 == l):
                # scal16 = E^T @ scaling
                scal_sb = preS.tile([L, 1], F32, tag="ssb")
                nc.sync.dma_start(out=scal_sb[:],
                                  in_=scal.rearrange("(p f) -> p f", f=1))
                lcol_i = preS.tile([L, 1], I32, tag="lci")
                nc.gpsimd.iota(lcol_i[:], pattern=[[0, 1]], base=0,
                               channel_multiplier=1)
                lcol_f = preS.tile([L, 1], F32, tag="lcf")
                nc.vector.tensor_copy(lcol_f[:], lcol_i[:])
                ibc8 = psS.tile([L, P], F32, tag="ibc8")
                nc.tensor.matmul(ibc8[:], ones_row[0:1, 0:L], irow_f[:],
                                 start=True, stop=True)
                e_sb = preS.tile([L, P], F32, tag="esb")
                nc.vector.tensor_scalar(e_sb[:], ibc8[:], lcol_f[:], None, EQ)
                s16ps = psS.tile([P, 1], F32, tag="s16ps")
                nc.tensor.matmul(s16ps[:], e_sb[:], scal_sb[:],
                                 start=True, stop=True)
                nc.vector.tensor_copy(scal16[:], s16ps[:])

                # ball[p, oc] = b[oc*128 + p] via 32 micro PE column
                # broadcasts of the bias row.
                brow = preS.tile([1, O], F32, tag="brow")
                nc.sync.dma_start(out=brow[:],
                                  in_=b.rearrange("(a f) -> a f", a=1))
                ball_ps = psS.tile([P, NOC], F32, tag="ballps")
                for oc in range(NOC):
                    nc.tensor.matmul(ball_ps[:, oc:oc + 1],
                                     brow[0:1, oc * P:(oc + 1) * P],
                                     ones_row[0:1, 0:1],
                                     start=True, stop=True)
                nc.vector.tensor_copy(ball[:], ball_ps[:])
                psS_cm.__exit__(None, None, None)

                # Phase W: adapters. wbsT: per 128-wide o-tile, one DMA
                # gathers [o=128, (l,r)=128] (issued on the scalar queue so
                # the sync queue stays free for the big streams), one PE
                # transpose flips to [(l,r), o], and the eviction folds in
                # scaling + bf16 cast.
                with (
                    tc.tile_pool(name="preW", bufs=2) as preW,
                    tc.tile_pool(name="psW", bufs=2, space="PSUM") as psW,
                ):
                    for ot in range(O // P):
                        nat = preW.tile([P, P], F32R, tag="wbnat")
                        src = wb[:, ot * P:(ot + 1) * P, :].transpose([1, 0, 2])
                        nc.scalar.dma_start(out=nat[:], in_=src)
                        pt = psW.tile([P, P], F32R, tag="wbps")
                        nc.tensor.transpose(pt[:], nat[:], ident[:])
                        nc.vector.tensor_scalar(wbsT[:, ot * P:(ot + 1) * P],
                                                pt[:], scal16[:], None, MUL)

                    # WA -> waT chunks [d, j]
                    for q in range(4):
                        wa_nat = preW.tile([P, 1024], F32R, tag="nat1k")
                        nc.scalar.dma_start(out=wa_nat[:],
                                            in_=wa[:, q * 1024:(q + 1) * 1024])
                        for half in range(2):
                            pt4 = psW.tile([P, 4 * P], F32R, tag="waps")
                            for k in range(4):
                                kk = half * 4 + k
                                nc.tensor.transpose(
                                    pt4[:, k * P:(k + 1) * P],
                                    wa_nat[:, kk * P:(kk + 1) * P],
                                    ident[:])
                            nc.any.tensor_copy(
                                waT[:, (q * 8 + half * 4) * P:
                                    (q * 8 + half * 4 + 4) * P], pt4[:])

                # token_lora rows (int -> float, 512-wide offset-0 tiles)
                tlfs = []
                for h in range(2):
                    tli = preS.tile([1, HT], I32, tag=f"tli{h}",
                                    name=f"tli{h}")
                    nc.sync.dma_start(
                        out=tli[:],
                        in_=tl[h * HT:(h + 1) * HT]
                        .rearrange("(a f) -> a f", a=1))
                    tlf = preS.tile([1, HT], F32, tag=f"tlf{h}",
                                    name=f"tlf{h}")
                    nc.vector.tensor_copy(tlf[:], tli[:])
                    tlfs.append(tlf)

                # Phase X: x -> xT (PE transposes, batched 4 per PSUM tile),
                # interleaved with the u = WA @ x^T accumulation so the dense
                # LoRA projection finishes with the transpose pass.
                with (
                    tc.tile_pool(name="preX", bufs=3) as preX,
                    tc.tile_pool(name="psX", bufs=3, space="PSUM") as psX,
                    tc.tile_pool(name="psU", bufs=1, space="PSUM") as psU,
                ):
                    u_ps = [psU.tile([P, HT], F32, tag=f"ups{h}",
                                     name=f"ups{h}") for h in range(2)]
                    for q in range(4):
                        for tt in range(NT):
                            x_nat = preX.tile([P, 1024], F32R, tag="nat1k")
                            nc.sync.dma_start(
                                out=x_nat[:],
                                in_=x[tt * P:(tt + 1) * P,
                                      q * 1024:(q + 1) * 1024])
                            for half in range(2):
                                pt4 = psX.tile([P, 4 * P], F32R, tag="xps")
                                for k in range(4):
                                    kk = half * 4 + k
                                    nc.tensor.transpose(
                                        pt4[:, k * P:(k + 1) * P],
                                        x_nat[:, kk * P:(kk + 1) * P],
                                        ident[:])
                                for k in range(4):
                                    dc = q * 8 + half * 4 + k
                                    nc.any.tensor_copy(
                                        xT[:, dc * TS + tt * P:
                                           dc * TS + (tt + 1) * P],
                                        pt4[:, k * P:(k + 1) * P])
                        for k in range(8):
                            dc = q * 8 + k
                            for h in range(2):
                                nc.tensor.matmul(
                                    u_ps[h][:],
                                    waT[:, dc * P:(dc + 1) * P],
                                    xT[:, dc * TS + h * HT:
                                       dc * TS + (h + 1) * HT],
                                    start=(dc == 0), stop=(dc == ND - 1))

                    # routing mask + bf16 eviction of the selected u
                    with tc.tile_pool(name="psM", bufs=2,
                                      space="PSUM") as psM:
                        for h in range(2):
                            tlbc = psM.tile([P, HT], F32, tag="tlbc")
                            nc.tensor.matmul(tlbc[:], ones_row[:],
                                             tlfs[h][:],
                                             start=True, stop=True)
                            maskT = preS.tile([P, HT], F32, tag=f"maskT{h}",
                                              name=f"maskT{h}")
                            nc.vector.tensor_scalar(maskT[:], tlbc[:],
                                                    jdiv16[:], None, EQ)
                            nc.vector.tensor_tensor(
                                u_mT[:, h * HT:(h + 1) * HT], u_ps[h][:],
                                maskT[:], MUL)

            # ---------------- main loop ----------------
            # Per oc: stream W rows, PE-transpose them (pipelined one oc
            # ahead), run the 32-chunk contraction + LoRA into PSUM, evict
            # with bias on the scalar engine, PE-transpose the output back
            # (pipelined one oc behind) and DMA out.
            with (
                tc.tile_pool(name="wnat", bufs=3) as wnat_p,
                tc.tile_pool(name="wt", bufs=2) as wt_p,
                tc.tile_pool(name="outT", bufs=2) as outT_p,
                tc.tile_pool(name="osb", bufs=6) as osb_p,
                tc.tile_pool(name="acc_ps", bufs=2, space="PSUM") as acc_ps,
                tc.tile_pool(name="wtr_ps", bufs=2, space="PSUM") as wtr_ps,
                tc.tile_pool(name="otr_ps", bufs=2, space="PSUM") as otr_ps,
            ):
                def stage_w(oc):
                    """DMA the W rows for o-chunk oc and transpose to bf16
                    [d, o] stationaries."""
                    wnat = wnat_p.tile([P, D], F32R, tag="wnat",
                                       name=f"wnat{oc}")
                    nc.sync.dma_start(out=wnat[:], in_=w[oc * P:(oc + 1) * P, :])
                    wt = wt_p.tile([P, ND * P], BF16, tag="wt",
                                   name=f"wt{oc}")
                    for batch in range(8):
                        wtr = wtr_ps.tile([P, 512], F32R, tag="wtr",
                                          name=f"wtr{oc}_{batch}")
                        for k in range(4):
                            dc = batch * 4 + k
                            nc.tensor.transpose(wtr[:, k * P:(k + 1) * P],
                                                wnat[:, dc * P:(dc + 1) * P],
                                                ident[:])
                        nc.any.tensor_copy(
                            wt[:, batch * 512:(batch + 1) * 512], wtr[:])
                    return wt

                def emit_out(oc, outT):
                    """Transpose outT back to [t, o] tiles and DMA them."""
                    for g in range(2):
                        otr = otr_ps.tile([P, 512], F32R, tag="otr",
                                          name=f"otr{oc}_{g}")
                        for k in range(4):
                            tt = g * 4 + k
                            nc.tensor.transpose(otr[:, k * P:(k + 1) * P],
                                                outT[:, tt * P:(tt + 1) * P],
                                                ident[:])
                        osb = osb_p.tile([P, 512], F32, tag="osb",
                                         name=f"osb{oc}_{g}")
                        nc.any.tensor_copy(osb[:], otr[:])
                        # one 3D DMA covers all 4 token tiles of this half
                        dst = out[g * 4 * P:(g + 1) * 4 * P,
                                  oc * P:(oc + 1) * P]\
                            .rearrange("(k t) o -> t k o", k=4)
                        nc.sync.dma_start(
                            out=dst,
                            in_=osb[:].rearrange("p (k o) -> p k o", k=4))

                wt_cur = stage_w(0)
                prev = None
                for oc in range(NOC):
                    wt_next = stage_w(oc + 1) if oc + 1 < NOC else None

                    accs = [acc_ps.tile([P, HT], F32, tag="acc",
                                        name=f"acc{oc}_{h}") for h in range(2)]
                    for dc in range(ND):
                        for h in range(2):
                            nc.tensor.matmul(
                                accs[h][:],
                                wt_cur[:, dc * P:(dc + 1) * P],
                                xT[:, dc * TS + h * HT: dc * TS + (h + 1) * HT],
                                start=(dc == 0), stop=False)
                    for h in range(2):
                        nc.tensor.matmul(
                            accs[h][:],
                            wbsT[:, oc * P:(oc + 1) * P],
                            u_mT[:, h * HT:(h + 1) * HT],
                            start=False, stop=True)

                    outT = outT_p.tile([P, TS], F32R, tag="outT",
                                       name=f"outT{oc}")
                    for h in range(2):
                        nc.scalar.activation(outT[:, h * HT:(h + 1) * HT],
                                             accs[h][:], IDENT,
                                             bias=ball[:, oc:oc + 1])

                    if prev is not None:
                        emit_out(*prev)
                    prev = (oc, outT)
                    wt_cur = wt_next
                emit_out(*prev)
    nc.finalize()
    return nc


_NC = None


def _get_nc():
    global _NC
    if _NC is None:
        _NC = _build()
    return _NC


class _Runner:
    """Cached PJRT executable for the SPMD bass kernel.

    Mirrors concourse.bass2jax.run_bass_via_pjrt's multi-core path but
    keeps the jitted shard_map callable alive across invocations so
    repeated kernel() calls skip retrace/recompile.
    """

    # inputs sharded over the token dim; everything else replicated
    _CORE_SHARDED = {"x", "tl"}

    def __init__(self):
        import jax
        import concourse.mybir as mybir_
        from concourse import bass2jax

        bass2jax.install_neuronx_cc_hook()
        self._bass2jax = bass2jax
        nc = _get_nc()
        self.nc = nc

        partition_name = (nc.partition_id_tensor.name
                          if nc.partition_id_tensor else None)
        in_names, out_names, out_avals, zero_outs = [], [], [], []
        for alloc in nc.m.functions[0].allocations:
            if not isinstance(alloc, mybir_.MemoryLocationSet):
                continue
            name = alloc.memorylocations[0].name
            if alloc.kind == "ExternalInput":
                if name != partition_name:
                    in_names.append(name)
            elif alloc.kind == "ExternalOutput":
                shape = tuple(alloc.tensor_shape)
                dtype = mybir_.dt.np(alloc.dtype)
                out_names.append(name)
                out_avals.append(jax.core.ShapedArray(shape, dtype))
                zero_outs.append((shape, dtype))
        self.in_names = list(in_names)
        self.out_names = out_names
        self.out_avals = out_avals
        n_params = len(in_names)
        all_in_names = in_names + out_names
        if partition_name is not None:
            all_in_names.append(partition_name)

        from jax.experimental.shard_map import shard_map
        from jax.sharding import Mesh, NamedSharding, PartitionSpec

        devices = jax.devices()[:NCORES]
        assert len(devices) == NCORES, devices
        mesh = Mesh(np.asarray(devices), ("core",))
        self.mesh = mesh

        def spec_for(name):
            return (PartitionSpec("core") if name in self._CORE_SHARDED
                    else PartitionSpec())

        in_specs = tuple(spec_for(n) for n in in_names) + \
            (PartitionSpec("core"),) * len(out_names)
        out_specs = (PartitionSpec("core"),) * len(out_names)
        self.in_shardings = [NamedSharding(mesh, spec_for(n))
                             for n in in_names]
        self.out_sharding = NamedSharding(mesh, PartitionSpec("core"))

        def _body(*args):
            operands = list(args)
            if partition_name is not None:
                operands.append(bass2jax.partition_id_tensor())
            outs = bass2jax._bass_exec_p.bind(
                *operands,
                out_avals=tuple(out_avals),
                in_names=tuple(all_in_names),
                out_names=tuple(out_names),
                lowering_input_output_aliases=(),
                sim_require_finite=True,
                sim_require_nnan=True,
                nc=nc,
            )
            return tuple(outs)

        self._fn = jax.jit(
            shard_map(_body, mesh=mesh, in_specs=in_specs,
                      out_specs=out_specs, check_rep=False),
            keep_unused=True)
        # resident zero operands for the NEFF's output-tensor inputs (the
        # kernel writes every output element, so contents don't matter and
        # the same device buffers are reused every call)
        import jax
        self._scratch_dev = [
            jax.device_put(
                np.zeros((NCORES * a.shape[0], *a.shape[1:]), a.dtype),
                self.out_sharding)
            for a in out_avals
        ]

    def put_inputs(self, by_name):
        import jax
        out = []
        for name, sharding in zip(self.in_names, self.in_shardings):
            out.append(jax.device_put(by_name[name], sharding))
        return out

    def run_device(self, dev_args):
        """dev_args: device arrays in in_names order. Returns jax arrays."""
        return self._fn(*dev_args, *self._scratch_dev)

    def run(self, by_name):
        outs = self.run_device(self.put_inputs(by_name))
        host = [np.asarray(o) for o in outs]
        return {n: h for n, h in zip(self.out_names, host)}


_RUNNER = None


def _get_runner():
    global _RUNNER
    if _RUNNER is None:
        _RUNNER = _Runner()
    return _RUNNER


def _global_inputs(x, W_base, b_base, WA, WB, scaling, token_lora):
    """Full-size (global) arrays keyed by DRAM-parameter name."""
    return {
        "x": np.ascontiguousarray(np.asarray(x, dtype=np.float32)),
        "w": np.ascontiguousarray(np.asarray(W_base, dtype=np.float32)),
        "b": np.ascontiguousarray(np.asarray(b_base, dtype=np.float32)),
        "wa": np.ascontiguousarray(
            np.asarray(WA, dtype=np.float32).reshape(LR, D)),
        "wb": np.ascontiguousarray(np.asarray(WB, dtype=np.float32)),
        "scal": np.ascontiguousarray(np.asarray(scaling, dtype=np.float32)),
        "tl": np.ascontiguousarray(np.asarray(token_lora, dtype=np.int32)),
    }


def kernel(x, W_base, b_base, WA, WB, scaling, token_lora):
    by_name = _global_inputs(x, W_base, b_base, WA, WB, scaling, token_lora)
    try:
        res = _get_runner().run(by_name)
        return res["out"]
    except Exception:
        # robust fallback through the library SPMD path
        from concourse.bass_utils import run_bass_kernel_spmd

        nc = _get_nc()
        in_maps = []
        for c in range(NCORES):
            in_maps.append({
                "x": by_name["x"][c * TS:(c + 1) * TS],
                "w": by_name["w"],
                "b": by_name["b"],
                "wa": by_name["wa"],
                "wb": by_name["wb"],
                "scal": by_name["scal"],
                "tl": by_name["tl"][c * TS:(c + 1) * TS],
            })
        res = run_bass_kernel_spmd(nc, in_maps, core_ids=list(range(NCORES)))
        return np.concatenate(
            [res.results[c]["out"] for c in range(NCORES)], axis=0)
